# revision 1
# baseline (speedup 1.0000x reference)
"""Trainium2 Bass kernel for the 2-layer GAT model (top-10 attention, 4 heads).

Strategy (8 NeuronCores, SPMD):
- Nodes sharded into 8 contiguous ranges of 6250 (dst ranges == GEMM shards).
- Within each core, dst nodes are degree-sorted into 49 tiles of 128 with a
  common per-tile slot count D[t] (shared across cores: one NEFF, 8 cores).
- The edge gather is DMA-transaction-bound (~6ns/descriptor regardless of
  256B..1KB size), so each edge-slot is fetched with ONE descriptor: the
  gather table row is [xl 256 fp16 | a_s 4 f32-as-bits | pad] = 384 fp16
  (768B), built by the GEMM phase and AllGathered (fp16) to all cores.
  a_d is destination-side and stays local (no asd AllGather at all).
- Edge phase per tile: gather folded rows in groups of CH=48 slots
  (j-major, int16 indices based at row 32768), extract a_s via f32 bitcast,
  alpha = leaky_relu(a_s+a_d), top-10 per (dst,head) via max8+match_replace,
  exn = exp(alpha-max)*masks/denom in fp16, multiply resident fp16 rows
  in-place, accumulate per-slot with fp16 identity matmuls into f32 PSUM.
- Layer-2 GEMM is fused per-tile into layer-1's edge loop (no DRAM round
  trip for out1); head-mean + 2-layer MLP fused per tile in layer 2.
"""
import numpy as np

N = 50000
E = 800000
F_IN = 256
H, C = 4, 64
HC = H * C
K_TOP = 10
NEG_SLOPE = 0.2
N_CORES = 8
SH = N // N_CORES            # 6250 real rows per core
TILES = (SH + 127) // 128    # 49
ROWS = TILES * 128           # 6272 padded rows per core
NT = N_CORES * ROWS          # 50176 global table rows
BASE = 32768                 # signed-int16 gather base row
PADROW = NT - 1              # ghost row of last core (content = f(zeros))
CH = 48                      # fold gather group (slots per dma_gather)
AGC = 2                      # AllGather row-chunks (chunk-major table layout)
TGROUPS = [25, 24]           # tiles per AG chunk (sum = TILES)
FW = 384                     # folded row width in fp16 elems (768B)
HID, OUT_F = 128, 16


def _wrap_idx(vals: np.ndarray) -> np.ndarray:
    """int16 index list -> [128, ceil(len/16)] wrapped+replicated tile."""
    ni = len(vals)
    w = -(-ni // 16)
    arr = np.full(w * 16, PADROW - BASE, np.int16)
    arr[:ni] = vals
    return np.tile(arr.reshape(w, 16).T, (8, 1))


def _prep(x, W1, att_s1, att_d1, W2, att_s2, att_d2, Wl1, Wl2, edge_index):
    """Host preprocessing: sharding, degree-sorted tiles, gather index tables."""
    src = np.asarray(edge_index[0], np.int64)
    dst = np.asarray(edge_index[1], np.int64)

    deg = np.bincount(dst, minlength=N)
    loc = np.empty(N, np.int64)
    node_of = np.full((N_CORES, ROWS), -1, np.int64)  # local row -> global node
    for c in range(N_CORES):
        nodes = np.arange(c * SH, (c + 1) * SH)
        order = np.argsort(-deg[nodes], kind="stable")
        loc[nodes[order]] = np.arange(SH)
        node_of[c, :SH] = nodes[order]
    # chunk-major table row: [chunk k][core c][local row within chunk]
    t0s = np.cumsum([0] + TGROUPS)            # tile offsets per chunk
    r0s = t0s * 128                           # row offsets per chunk
    Bks = np.cumsum([0] + [N_CORES * g * 128 for g in TGROUPS])
    core_of = np.arange(N) // SH
    kk = np.searchsorted(r0s, loc, side="right") - 1
    Rk = (np.array(TGROUPS) * 128)[kk]
    rowid = Bks[kk] + core_of * Rk + (loc - r0s[kk])  # node -> table row

    # common D schedule
    degl = np.zeros((N_CORES, ROWS), np.int64)
    for c in range(N_CORES):
        degl[c, :SH] = deg[node_of[c, :SH]]
    tile_max = degl.reshape(N_CORES, TILES, 128).max(axis=(0, 2))
    D = np.maximum(8, ((tile_max + 3) // 4) * 4).astype(np.int64)
    assert D.max() <= 2 * CH + 32, f"degree too high for this kernel: {D.max()}"

    # CSR of edges by (core, local dst row)
    e_loc = (dst // SH) * ROWS + loc[dst]
    order_e = np.argsort(e_loc, kind="stable")
    src_s = src[order_e]
    e_loc_s = e_loc[order_e]
    starts = np.searchsorted(e_loc_s, np.arange(N_CORES * ROWS))
    ends = np.searchsorted(e_loc_s, np.arange(N_CORES * ROWS) + 1)

    PAD16 = np.int16(PADROW - BASE)
    idx_xf_parts = [[] for _ in range(N_CORES)]
    chunks = []  # per tile: list of (j0, jc)
    for t in range(TILES):
        Dt = int(D[t])
        chunks.append([(j0, min(CH, Dt - j0)) for j0 in range(0, Dt, CH)])
    for c in range(N_CORES):
        for t in range(TILES):
            Dt = int(D[t])
            slot = np.full((128, Dt), PADROW, np.int64)
            for d in range(128):
                r = c * ROWS + t * 128 + d
                s, e = starts[r], ends[r]
                if e > s:
                    slot[d, : e - s] = rowid[src_s[s:e]]
            s16 = (slot - BASE).astype(np.int16)
            for (j0, jc) in chunks[t]:
                part = s16[:, j0 : j0 + jc].T.reshape(-1)
                idx_xf_parts[c].append(_wrap_idx(np.concatenate([part, [PAD16]])))

    idx_xf = np.stack([np.concatenate(p, axis=1) for p in idx_xf_parts])

    degf = np.zeros((N_CORES, 128, TILES), np.float32)
    for c in range(N_CORES):
        degf[c] = degl[c].reshape(TILES, 128).T.astype(np.float32)

    x_shard = np.zeros((N_CORES, ROWS, F_IN), np.float32)
    for c in range(N_CORES):
        x_shard[c, :SH] = np.asarray(x)[node_of[c, :SH]]

    def att_fold(WT, att_s, att_d):
        Vs = np.stack([WT[:, h * C : (h + 1) * C] @ np.asarray(att_s)[0, h]
                       for h in range(H)], axis=1)
        Vd = np.stack([WT[:, h * C : (h + 1) * C] @ np.asarray(att_d)[0, h]
                       for h in range(H)], axis=1)
        return np.hstack([WT, Vs, Vd]).astype(np.float32)

    W1T_ext = att_fold(np.asarray(W1).T.astype(np.float32), att_s1, att_d1)
    W2T_ext = att_fold(np.asarray(W2).T.astype(np.float32), att_s2, att_d2)

    meta = dict(D=[int(d) for d in D], chunks=chunks)
    consts = dict(
        W1T_ext=W1T_ext, W2T_ext=W2T_ext,
        Wl1T=np.asarray(Wl1).T.astype(np.float32).copy(),
        Wl2T=np.asarray(Wl2).T.astype(np.float32).copy(),
    )
    per_core = dict(x_shard=x_shard, idx_xf=idx_xf, degf=degf)
    return meta, consts, per_core, node_of


def build_gnn(meta, stage=4, sub=99, repeat=1, skip_ag=False, sp=False):
    from concourse import bass, bacc, mybir
    import concourse.tile as tile
    from concourse.masks import make_identity

    D = meta["D"]
    chunks = meta["chunks"]
    WXF = sum(-(-(128 * jc + 1) // 16) for t in range(TILES) for (_, jc) in chunks[t])

    f32 = mybir.dt.float32
    f16 = mybir.dt.float16
    i16 = mybir.dt.int16
    nc = bacc.Bacc(None, target_bir_lowering=False, num_devices=N_CORES,
                   num_swdge_queues=4)

    # inputs
    x_in = nc.dram_tensor("x_shard", [ROWS, F_IN], f32, kind="ExternalInput")
    w1_in = nc.dram_tensor("W1T_ext", [F_IN, HC + 8], f32, kind="ExternalInput")
    w2_in = nc.dram_tensor("W2T_ext", [HC, HC + 8], f32, kind="ExternalInput")
    wl1_in = nc.dram_tensor("Wl1T", [C, HID], f32, kind="ExternalInput")
    wl2_in = nc.dram_tensor("Wl2T", [HID, OUT_F], f32, kind="ExternalInput")
    bl1_in = nc.dram_tensor("bl1_col", [HID, 1], f32, kind="ExternalInput")
    bl2_in = nc.dram_tensor("bl2_rep", [128, OUT_F], f32, kind="ExternalInput")
    b1_in = nc.dram_tensor("b1_rep", [128, HC], f32, kind="ExternalInput")
    b2_in = nc.dram_tensor("b2_rep", [128, C], f32, kind="ExternalInput")
    dg_in = nc.dram_tensor("degf", [128, TILES], f32, kind="ExternalInput")
    ix_in = nc.dram_tensor("idx_xf", [128, WXF], i16, kind="ExternalInput")

    out_dram = nc.dram_tensor("out", [ROWS, OUT_F], f32, kind="ExternalOutput")

    # internal DRAM: folded fp16 tables (row: xl 256 f16 | a_s 4 f32bits | pad)
    xf_sh = [nc.dram_tensor(f"xf{l}_shard", [ROWS, FW], f16) for l in (1, 2)]
    xf_fu = [nc.dram_tensor(f"xf{l}_full", [NT, FW], f16, addr_space="Shared")
             for l in (1, 2)]
    asd_lo = [nc.dram_tensor(f"asd{l}_local", [ROWS, 8], f32) for l in (1, 2)]

    LR = mybir.ActivationFunctionType.Lrelu
    CPY = mybir.ActivationFunctionType.Copy
    MAXO = mybir.AluOpType.max
    EXP = mybir.ActivationFunctionType.Exp
    RELU = mybir.ActivationFunctionType.Relu
    ADD = mybir.AluOpType.add
    MUL = mybir.AluOpType.mult
    SUB = mybir.AluOpType.subtract
    GE = mybir.AluOpType.is_ge
    LT = mybir.AluOpType.is_lt

    with tile.TileContext(nc) as tc:
        with (
            tc.tile_pool(name="const", bufs=1) as cpool,
            tc.tile_pool(name="gemm", bufs=2) as gpool,
            tc.tile_pool(name="gpsum", bufs=1, space="PSUM") as gpsum,
            tc.tile_pool(name="fold", bufs=3) as fpool,
            tc.tile_pool(name="idxp", bufs=3) as ipool,
            tc.tile_pool(name="mid", bufs=2) as mpool,
            tc.tile_pool(name="work", bufs=3) as wpool,
            tc.tile_pool(name="small", bufs=8) as spool,
            tc.tile_pool(name="agg", bufs=2, space="PSUM") as apsum,
            tc.tile_pool(name="mlpp", bufs=2, space="PSUM") as mpsum,
        ):
            # ---- constants ----
            ident = cpool.tile([128, 128], f32)
            make_identity(nc, ident[:])
            ident16 = cpool.tile([128, 128], f16)
            nc.vector.tensor_copy(ident16[:], ident[:])
            iota_i = cpool.tile([128, 128], mybir.dt.int32)
            nc.gpsimd.iota(iota_i[:], pattern=[[1, 128]], base=0, channel_multiplier=0)
            iota_f = cpool.tile([128, 128], f32)
            nc.vector.tensor_copy(iota_f[:], iota_i[:])
            w1_sb = cpool.tile([128, 2, HC + 8], f32)
            nc.sync.dma_start(out=w1_sb[:, 0], in_=w1_in[0:128])
            nc.sync.dma_start(out=w1_sb[:, 1], in_=w1_in[128:256])
            w2_sb = cpool.tile([128, 2, HC + 8], f32)
            nc.sync.dma_start(out=w2_sb[:, 0], in_=w2_in[0:128])
            nc.sync.dma_start(out=w2_sb[:, 1], in_=w2_in[128:256])
            wl1_sb = cpool.tile([C, HID], f32)
            nc.sync.dma_start(out=wl1_sb[:], in_=wl1_in[:])
            wl2_sb = cpool.tile([HID, OUT_F], f32)
            nc.sync.dma_start(out=wl2_sb[:], in_=wl2_in[:])
            bl1_sb = cpool.tile([HID, 1], f32)
            nc.sync.dma_start(out=bl1_sb[:], in_=bl1_in[:])
            bl2_sb = cpool.tile([128, OUT_F], f32)
            nc.sync.dma_start(out=bl2_sb[:], in_=bl2_in[:])
            b1_sb = cpool.tile([128, HC], f32)
            nc.sync.dma_start(out=b1_sb[:], in_=b1_in[:])
            b2_sb = cpool.tile([128, C], f32)
            nc.sync.dma_start(out=b2_sb[:], in_=b2_in[:])
            deg_sb = cpool.tile([128, TILES], f32)
            nc.sync.dma_start(out=deg_sb[:], in_=dg_in[:])
            wl1_16 = cpool.tile([C, HID], f16)
            nc.vector.tensor_copy(wl1_16[:], wl1_sb[:])
            wl2_16 = cpool.tile([HID, OUT_F], f16)
            nc.vector.tensor_copy(wl2_16[:], wl2_sb[:])
            o2b = cpool.tile([C, ROWS], f16)

            qrr = [0]

            def gemm_tile(xT, w_sb, l, rows):
                """xT: [128, 2, 128] transposed input halves -> fold row out."""
                li = l - 1
                ps = gpsum.tile([128, HC + 8], f32, tag="g_mm")
                nc.tensor.matmul(ps[:], xT[:, 0], w_sb[:, 0], start=True, stop=False)
                nc.tensor.matmul(ps[:], xT[:, 1], w_sb[:, 1], start=False, stop=True)
                og = gpool.tile([128, FW], f16, tag="g_out")
                nc.vector.tensor_copy(og[:, :HC], ps[:, :HC])
                nc.vector.tensor_copy(
                    og[:, HC : HC + 8].bitcast(f32), ps[:, HC : HC + 4]
                )
                oga = gpool.tile([128, 8], f32, tag="g_oa")
                nc.vector.tensor_copy(oga[:], ps[:, HC : HC + 8])
                nc.sync.dma_start(out=xf_sh[li][rows], in_=og[:])
                nc.sync.dma_start(out=asd_lo[li][rows], in_=oga[:])

            def gemm1_phase():
                for t in range(TILES):
                    rows = slice(t * 128, (t + 1) * 128)
                    xt = gpool.tile([128, F_IN], f32, tag="g_in")
                    nc.sync.dma_start(out=xt[:], in_=x_in[rows])
                    xT = gpool.tile([128, 2, 128], f32, tag="g_T")
                    for k in range(2):
                        pst = gpsum.tile([128, 128], f32, tag="g_tp")
                        nc.tensor.transpose(pst[:], xt[:, k * 128 : (k + 1) * 128], ident[:])
                        nc.vector.tensor_copy(xT[:, k], pst[:])
                    gemm_tile(xT, w1_sb, 1, rows)
                    if not skip_ag and stage >= 1 and (t + 1) in t0s:
                        allgather_chunk(1, t0s.index(t + 1) - 1)

            t0s = [0]
            for g in TGROUPS:
                t0s.append(t0s[-1] + g)
            bks = [0]
            for g in TGROUPS:
                bks.append(bks[-1] + N_CORES * g * 128)

            def allgather_chunk(l, k):
                li = l - 1
                rs_in = xf_sh[li].ap()[t0s[k] * 128 : t0s[k + 1] * 128]
                rs_out = xf_fu[li].ap()[bks[k] : bks[k + 1]]
                nc.gpsimd.collective_compute(
                    "AllGather", mybir.AluOpType.bypass,
                    replica_groups=[list(range(N_CORES))],
                    ins=[rs_in.opt()], outs=[rs_out.opt()],
                )

            def edge_phase(l, sub=99):
                li = l - 1
                table = xf_fu[li]
                adr = asd_lo[li].ap().rearrange("(t d) c -> d t c", d=128)
                ad_all = cpool.tile([128, TILES, 4], f32, tag=f"ad{l}")
                nc.sync.dma_start(out=ad_all[:], in_=adr[:, :, 4:8])

                oxf = 0
                for t in range(TILES):
                    Dt = D[t]
                    alpha = mpool.tile([128, H, Dt], f32, tag="alpha")
                    folds = []
                    for (j0, jc) in chunks[t]:
                        w = -(-(128 * jc + 1) // 16)
                        ixf = ipool.tile([128, w], i16, tag="ixf")
                        nc.sync.dma_start(out=ixf[:], in_=ix_in[:, oxf : oxf + w])
                        oxf += w
                        fg = fpool.tile([128, jc + 1, FW], f16, tag="fold")
                        nc.gpsimd.dma_gather(
                            out_ap=fg[:], in_ap=table[BASE:, :], idxs_ap=ixf[:],
                            num_idxs=128 * jc + 1, num_idxs_reg=128 * jc + 1,
                            elem_size=FW, single_packet=sp, queue_num=qrr[0] % 4,
                        )
                        qrr[0] += 1
                        folds.append((j0, jc, fg))
                        if sub == 25:
                            dmy = spool.tile([128, 16], f16, tag="dmy")
                            nc.vector.tensor_copy(dmy[:], fg[:, 0, 0:16])
                            dmyf = spool.tile([128, 16], f32, tag="dmyf")
                            nc.vector.tensor_copy(dmyf[:], dmy[:])
                            nc.sync.dma_start(
                                out=out_dram[t * 128 : (t + 1) * 128, 0:16],
                                in_=dmyf[:])
                            continue
                        nc.vector.tensor_tensor(
                            out=alpha[:, :, j0 : j0 + jc],
                            in0=fg[:, :jc, HC : HC + 8].bitcast(f32)[:, :, 0:4]
                                .transpose([0, 2, 1]),
                            in1=ad_all[:, t].unsqueeze(2).broadcast_to([128, H, jc]),
                            op=ADD,
                        )
                    if sub == 25:
                        continue
                    # alpha = leaky_relu(alpha); pad slots -> -1e30
                    lt1 = mpool.tile([128, H, Dt], f32, tag="lt1")
                    nc.scalar.activation(lt1[:], alpha[:], CPY, scale=NEG_SLOPE)
                    nc.vector.tensor_tensor(out=alpha[:], in0=alpha[:], in1=lt1[:], op=MAXO)
                    pm = spool.tile([128, Dt], f32, tag="pm")
                    nc.vector.tensor_scalar(
                        out=pm[:], in0=iota_f[:, :Dt],
                        scalar1=deg_sb[:, t : t + 1], scalar2=None, op0=LT,
                    )
                    pb = spool.tile([128, Dt], f32, tag="pb")
                    nc.vector.tensor_scalar(
                        out=pb[:], in0=pm[:], scalar1=1.0, scalar2=1e30,
                        op0=SUB, op1=MUL,
                    )
                    nc.vector.tensor_tensor(
                        out=alpha[:], in0=alpha[:],
                        in1=pb[:].unsqueeze(1).broadcast_to([128, H, Dt]), op=ADD,
                    )
                    if sub <= 20:
                        dt_ = spool.tile([128, H], f32, tag="dbga")
                        nc.vector.tensor_copy(dt_[:], alpha[:, :, 0])
                        nc.sync.dma_start(out=out_dram[t * 128 : t * 128 + 128, 0:H], in_=dt_[:])
                        continue
                    # --- topk threshold + row max ---
                    m_all = spool.tile([128, H], f32, tag="m_all")
                    t10 = spool.tile([128, H], f32, tag="t10")
                    if Dt > K_TOP:
                        for h in range(H):
                            m8 = spool.tile([128, 8], f32, tag="m8")
                            nc.vector.max(out=m8[:], in_=alpha[:, h])
                            nc.vector.tensor_copy(m_all[:, h : h + 1], m8[:, 0:1])
                            wk = mpool.tile([128, Dt], f32, tag="wk")
                            nc.vector.match_replace(
                                out=wk[:], in_to_replace=m8[:],
                                in_values=alpha[:, h], imm_value=-3e30,
                            )
                            m8b = spool.tile([128, 8], f32, tag="m8b")
                            nc.vector.max(out=m8b[:], in_=wk[:])
                            nc.vector.tensor_copy(t10[:, h : h + 1], m8b[:, 1:2])
                    else:
                        nc.vector.reduce_max(out=m_all[:], in_=alpha[:], axis=mybir.AxisListType.X)
                        nc.vector.memset(t10[:], -1e31)
                    # --- ex = exp(alpha - m) * topk_mask * padmask ---
                    ex = mpool.tile([128, H, Dt], f32, tag="ex")
                    nc.vector.tensor_tensor(
                        out=ex[:], in0=alpha[:],
                        in1=m_all[:].unsqueeze(2).broadcast_to([128, H, Dt]), op=SUB,
                    )
                    nc.scalar.activation(ex[:], ex[:], EXP)
                    msk = mpool.tile([128, H, Dt], f32, tag="msk")
                    nc.vector.tensor_tensor(
                        out=msk[:], in0=alpha[:],
                        in1=t10[:].unsqueeze(2).broadcast_to([128, H, Dt]), op=GE,
                    )
                    nc.vector.tensor_tensor(out=ex[:], in0=ex[:], in1=msk[:], op=MUL)
                    nc.vector.tensor_tensor(
                        out=ex[:], in0=ex[:],
                        in1=pm[:].unsqueeze(1).broadcast_to([128, H, Dt]), op=MUL,
                    )
                    # --- denom, normalized fp16 weights ---
                    den = spool.tile([128, H], f32, tag="den")
                    nc.vector.reduce_sum(out=den[:], in_=ex[:], axis=mybir.AxisListType.X)
                    nc.vector.tensor_scalar_max(den[:], den[:], 1e-20)
                    inv = spool.tile([128, H], f32, tag="inv")
                    nc.vector.reciprocal(inv[:], den[:])
                    if l == 2:
                        nc.vector.tensor_scalar_mul(inv[:], inv[:], 1.0 / H)
                    exn = mpool.tile([128, H, Dt], f16, tag="exn")
                    nc.vector.tensor_tensor(
                        out=exn[:], in0=ex[:],
                        in1=inv[:].unsqueeze(2).broadcast_to([128, H, Dt]), op=MUL,
                    )
                    if sub <= 21:
                        nc.sync.dma_start(out=out_dram[t * 128 : t * 128 + 128, 0:H], in_=inv[:])
                        nc.sync.dma_start(out=out_dram[t * 128 : t * 128 + 128, 4:8], in_=t10[:])
                        nc.sync.dma_start(out=out_dram[t * 128 : t * 128 + 128, 8:12], in_=m_all[:])
                        nc.sync.dma_start(out=out_dram[t * 128 : t * 128 + 128, 12:16], in_=den[:])
                        continue
                    # --- weight + identity-matmul accumulate ---
                    ps = apsum.tile([128, HC], f32, tag="agg")
                    for (j0, jc, fg) in folds:
                        nc.vector.tensor_tensor(
                            out=fg[:, :jc, :HC].rearrange("p j (h c) -> p j h c", h=H),
                            in0=fg[:, :jc, :HC].rearrange("p j (h c) -> p j h c", h=H),
                            in1=exn[:, :, j0 : j0 + jc].transpose([0, 2, 1])
                                .unsqueeze(3).broadcast_to([128, jc, H, C]),
                            op=MUL,
                        )
                        if sub == 26:
                            dmy = spool.tile([128, 16], f16, tag="dmy")
                            nc.vector.tensor_copy(dmy[:], fg[:, 0, 0:16])
                            dmyf = spool.tile([128, 16], f32, tag="dmyf")
                            nc.vector.tensor_copy(dmyf[:], dmy[:])
                            nc.sync.dma_start(
                                out=out_dram[t * 128 : (t + 1) * 128, 0:16],
                                in_=dmyf[:])
                            continue
                        for j in range(jc):
                            nc.tensor.matmul(
                                ps[:], ident16[:], fg[:, j, :HC],
                                start=(j0 + j == 0), stop=(j0 + j == Dt - 1),
                            )
                    if sub == 26:
                        continue
                    # --- finish tile ---
                    rows = slice(t * 128, (t + 1) * 128)
                    if l == 1:
                        o = wpool.tile([128, HC], f32, tag="o1")
                        nc.vector.tensor_tensor(out=o[:], in0=ps[:], in1=b1_sb[:], op=ADD)
                        if stage >= 3:
                            # fused layer-2 GEMM on the SBUF-resident tile
                            xT = gpool.tile([128, 2, 128], f32, tag="g_T")
                            for k in range(2):
                                pst = gpsum.tile([128, 128], f32, tag="g_tp")
                                nc.tensor.transpose(pst[:], o[:, k * 128 : (k + 1) * 128], ident[:])
                                nc.vector.tensor_copy(xT[:, k], pst[:])
                            gemm_tile(xT, w2_sb, 2, rows)
                            if not skip_ag and (t + 1) in t0s:
                                allgather_chunk(2, t0s.index(t + 1) - 1)
                        else:
                            nc.sync.dma_start(out=out_dram[rows], in_=o[:, :OUT_F])
                    else:
                        tmp = wpool.tile([128, HC], f32, tag="tmp2")
                        nc.vector.tensor_copy(tmp[:], ps[:])
                        o2 = spool.tile([128, C], f32, tag="o2")
                        nc.vector.reduce_sum(
                            out=o2[:],
                            in_=tmp[:].rearrange("p (h c) -> p c h", h=H),
                            axis=mybir.AxisListType.X,
                        )
                        nc.vector.tensor_tensor(out=o2[:], in0=o2[:], in1=b2_sb[:], op=ADD)
                        # stash transposed fp16 tile for the batched MLP tail
                        psT = mpsum.tile([C, 128], f32, tag="m_th")
                        nc.tensor.transpose(psT[:], o2[:], ident[:])
                        nc.vector.tensor_copy(o2b[:, t * 128 : (t + 1) * 128], psT[:])

            def mlp_tail():
                MC = 512
                cols = [(c0, min(MC, ROWS - c0)) for c0 in range(0, ROWS, MC)]
                for (c0, cw) in cols:
                    psh = mpsum.tile([HID, MC], f32, tag="m_big")
                    nc.tensor.matmul(psh[:, :cw], wl1_16[:], o2b[:, c0 : c0 + cw],
                                     start=True, stop=True)
                    rh = wpool.tile([HID, MC], f16, tag="rh")
                    nc.scalar.activation(rh[:, :cw], psh[:, :cw], RELU, bias=bl1_sb[:])
                    pso = mpsum.tile([OUT_F, MC], f32, tag="m_big")
                    nc.tensor.matmul(pso[:, :cw], wl2_16[:], rh[:, :cw],
                                     start=True, stop=True)
                    po = wpool.tile([OUT_F, MC], f32, tag="po")
                    nc.vector.tensor_copy(po[:, :cw], pso[:, :cw])
                    for b0 in range(0, cw, 128):
                        psf = mpsum.tile([128, OUT_F], f32, tag="m_th")
                        nc.tensor.transpose(psf[:], po[:, b0 : b0 + 128],
                                            ident[:OUT_F, :OUT_F])
                        of = spool.tile([128, OUT_F], f32, tag="of")
                        nc.vector.tensor_tensor(out=of[:], in0=psf[:], in1=bl2_sb[:], op=ADD)
                        nc.sync.dma_start(
                            out=out_dram[c0 + b0 : c0 + b0 + 128], in_=of[:])

            for _rep in range(repeat):
                gemm1_phase()
                if stage >= 2:
                    edge_phase(1, sub=sub)
                if stage >= 4:
                    edge_phase(2)
                    mlp_tail()
            if stage < 2 or (stage < 4 and sub <= 26):
                # debug readout so nothing is dead code
                src16 = xf_sh[0] if stage < 3 else xf_sh[1]
                dt16 = spool.tile([128, OUT_F], f16, tag="dbg16")
                nc.sync.dma_start(out=dt16[:], in_=src16[0:128, 0:OUT_F])
                dtile = spool.tile([128, OUT_F], f32, tag="dbg")
                nc.vector.tensor_copy(dtile[:], dt16[:])
                nc.sync.dma_start(out=out_dram[0:128], in_=dtile[:])

    nc.compile()
    return nc


def _make_in_maps(consts, per_core, b1, b2, bl1, bl2):
    b1 = np.asarray(b1, np.float32)
    b2 = np.asarray(b2, np.float32)
    bl1 = np.asarray(bl1, np.float32)
    bl2 = np.asarray(bl2, np.float32)
    shared = dict(
        W1T_ext=consts["W1T_ext"], W2T_ext=consts["W2T_ext"],
        Wl1T=consts["Wl1T"], Wl2T=consts["Wl2T"],
        bl1_col=np.ascontiguousarray(bl1[:, None]),
        bl2_rep=np.tile(bl2[None, :], (128, 1)),
        b1_rep=np.tile(b1[None, :], (128, 1)),
        b2_rep=np.tile(b2[None, :], (128, 1)),
    )
    return [
        dict(
            shared,
            x_shard=np.ascontiguousarray(per_core["x_shard"][c]),
            idx_xf=np.ascontiguousarray(per_core["idx_xf"][c]),
            degf=np.ascontiguousarray(per_core["degf"][c]),
        )
        for c in range(N_CORES)
    ]


def _assemble(results, node_of):
    out = np.empty((N, OUT_F), np.float32)
    for c in range(N_CORES):
        out[node_of[c, :SH]] = results[c]["out"][:SH]
    return out


def kernel(x, W1, att_s1, att_d1, b1, W2, att_s2, att_d2, b2,
           Wl1, bl1, Wl2, bl2, edge_index):
    from concourse.bass_utils import run_bass_kernel_spmd

    meta, consts, per_core, node_of = _prep(
        x, W1, att_s1, att_d1, W2, att_s2, att_d2, Wl1, Wl2, edge_index
    )
    nc = build_gnn(meta)
    in_maps = _make_in_maps(consts, per_core, b1, b2, bl1, bl2)
    res = run_bass_kernel_spmd(nc, in_maps, core_ids=list(range(N_CORES)))
    return _assemble(res.results, node_of)

